# revision 15
# baseline (speedup 1.0000x reference)
"""Dilated attention kernel for Trainium2 (8 NeuronCores, SPMD).

Problem: B=4, H=8, L=2048, D=128, dilation ratios [1,2,4,8].
Inputs  query/key/value: [32, 2048, 128] f32 (grouped (b h)).
Output: [4, 2048, 1024] f32 (b, l, h*d).

Math: for ratio dr, head h attends within the strided position subset
{p : p % dr == r}, r = h // (H//dr); results are scatter-added over ratios.

Key trick: permute positions by sigma(p) = rev3(p%8)*256 + p//8 (bit-reversal
of the low 3 bits moved to the top). Under sigma, every (dr, r) gather set
becomes a CONTIGUOUS row block, and the within-block order induced by sigma is
consistent across q/k/v and the output. So on-device everything is dense
attention over static row ranges; all gather/scatter is plain row permutation
done host-side during shard packing.

Sharding: core c = (batch b=c//2, query-half qh=c%2). Each core processes all
8 heads of its batch: the head loop (and thus the r-dependent block offsets)
is compile-time static, so one Bass program serves all 8 cores (SPMD).
Queries/outputs are split in half along the block rows; keys/values are full
per block. The host sums the per-ratio output blocks (they overlap across
ratios) and inverts sigma.

Shard layout prep (host side, per core): q and k are shipped pre-transposed
to [d, row] (the layout the PE contraction needs), v as bf16. On device each
head is then: S^T = K Q^T (float32r matmuls), exp on ScalarE (PSUM -> bf16
P^T tiles), O = P^T.T @ [V | 1] in bf16 (the ones column yields softmax row
sums for free), normalize with a per-partition reciprocal multiply.
"""

import numpy as np

B, H, L, D = 4, 8, 2048, 128
DRS = [1, 2, 4, 8]
REV3 = [0, 4, 2, 6, 1, 5, 3, 7]
# packed q/out row layout per head: ratio dr's query-half block lives at POFF[dr]
POFF = {1: 0, 2: 1024, 4: 1536, 8: 1792}
QROWS = 1920  # 1024 + 512 + 256 + 128

# sigma and its inverse as row-index arrays
P_OF_PI = np.array([(pi % 256) * 8 + REV3[pi // 256] for pi in range(L)])
SIG = np.empty(L, np.int64)
SIG[P_OF_PI] = np.arange(L)


def _rev(x, nbits):
    r = 0
    for i in range(nbits):
        r |= ((x >> i) & 1) << (nbits - 1 - i)
    return r


def _off(dr, h):
    """sigma-space row offset of the (dr, r(h)) block."""
    ld = dr.bit_length() - 1
    r = h >> (3 - ld)
    return _rev(r, ld) * (L // dr)


_CACHE = {}

# build-time tuning knobs (sweepable via sim)
CFG = {
    "strip": 512,      # l-strip width of the S phase (512 = 1 PSUM bank)
    "mc_pair": 2,      # m-chunks exp'd per activation op (psS = pair*1 banks)
    "ps_o_bufs": 4,
    "ps_s_bufs": 2,
    "sw_pipe": True,   # emit S(i+1) before PV(i)
}


def _build():
    """Build + compile the SPMD Bass program (identical on all 8 cores)."""
    import concourse.bass as bass
    import concourse.mybir as mybir
    import concourse.tile as tile
    from concourse import bacc

    f32 = mybir.dt.float32
    f32r = mybir.dt.float32r
    bf16 = mybir.dt.bfloat16

    nc = bacc.Bacc()
    qt = nc.dram_tensor("qt", [H, D, QROWS], f32r, kind="ExternalInput")
    kt = nc.dram_tensor("kt", [H, D, L], f32r, kind="ExternalInput")
    vb = nc.dram_tensor("vb", [H, L, D], bf16, kind="ExternalInput")
    o = nc.dram_tensor("o", [H, QROWS, D], f32, kind="ExternalOutput")

    NQ = QROWS // 128  # 15 chunks of packed q rows
    NK = L // 128      # 16 chunks of sigma-ordered k/v rows
    PAIR = CFG["mc_pair"]

    with tile.TileContext(nc) as tc:
        with (
            tc.tile_pool(name="singles", bufs=1) as singles,
            tc.tile_pool(name="work", bufs=2) as work,
            tc.tile_pool(name="pt_pool", bufs=2) as pt_pool,
            tc.tile_pool(name="small", bufs=8) as small,
            tc.tile_pool(name="ps_s", bufs=CFG["ps_s_bufs"], space="PSUM") as ps_s,
            tc.tile_pool(name="ps_o", bufs=CFG["ps_o_bufs"], space="PSUM") as ps_o,
        ):
            # constant bias for exp(s - 20): keeps exp values comfortably in
            # fp32/bf16 range without a data-dependent row max (|s| <= ~70)
            exp_bias = singles.tile([128, 1], f32)
            nc.vector.memset(exp_bias, -20.0)

            for h in range(H):
                # ---- load; q/k arrive pre-transposed [d, row] from host ----
                QT = work.tile([128, NQ, 128], f32r, tag="QT")
                nc.sync.dma_start(out=QT.rearrange("d c l -> d (c l)"), in_=qt[h])
                KT = work.tile([128, NK, 128], f32r, tag="KT")
                nc.sync.dma_start(out=KT.rearrange("d c l -> d (c l)"), in_=kt[h])
                # v in bf16 with a ones column (gives row sums in PV)
                vbf = work.tile([128, NK, 132], bf16, tag="vbf")
                nc.sync.dma_start(
                    out=vbf[:, :, 0:128],
                    in_=vb[h].rearrange("(c p) d -> p c d", p=128),
                )
                nc.vector.memset(vbf[:, :, 128:129], 1.0)

                ostage = work.tile([128, NQ, 128], f32, tag="ostage")

                # ---- per-ratio task list: (S-phase emit, PV-phase emit) ----
                def make_task(dr, strip, PTs):
                    Lg = L // dr
                    nM = Lg // 128
                    kc0 = _off(dr, h) // 128
                    qc0 = POFF[dr] // 128
                    ls = min(CFG["strip"], Lg // 2 - strip)
                    nls = ls // 128
                    sc0 = qc0 + strip // 128
                    PT = PTs

                    def s_phase():
                        for mc0 in range(0, nM, PAIR):
                            np_ = min(PAIR, nM - mc0)
                            psS = ps_s.tile([128, PAIR, 512], f32, tag="psS")
                            for i in range(np_):
                                # float32r: PE pseudo-fp32 (bf16 hi/lo dual
                                # pass), 1 cyc/row at N>=256 vs 4 for fp32
                                nc.tensor.matmul(
                                    psS[:, i, 0:ls],
                                    lhsT=KT[:, kc0 + mc0 + i, :],
                                    rhs=QT[:, sc0 : sc0 + nls, :],
                                    start=True,
                                    stop=True,
                                )
                            nc.scalar.activation(
                                out=PT[:, mc0 : mc0 + np_, :],
                                in_=psS[:, 0:np_, 0:ls],
                                func=mybir.ActivationFunctionType.Exp,
                                bias=exp_bias,
                                scale=1.0,
                            )

                    def pv_phase():
                        for lc in range(nls):
                            psO = ps_o.tile([128, 132], f32, tag="psO")
                            for mc in range(nM):
                                nc.tensor.matmul(
                                    psO[:, 0:129],
                                    lhsT=PT[:, mc, lc * 128 : (lc + 1) * 128],
                                    rhs=vbf[:, kc0 + mc, 0:129],
                                    start=(mc == 0),
                                    stop=(mc == nM - 1),
                                )
                            rec = small.tile([128, 1], f32, tag="rec")
                            nc.vector.reciprocal(rec, psO[:, 128:129])
                            nc.vector.tensor_scalar_mul(
                                ostage[:, sc0 + lc, :], psO[:, 0:128], rec
                            )

                    return s_phase, pv_phase

                tasks = []
                for dr in DRS:
                    Lg = L // dr
                    for strip in range(0, Lg // 2, CFG["strip"]):
                        ls = min(CFG["strip"], Lg // 2 - strip)
                        PT = pt_pool.tile(
                            [128, Lg // 128, ls], bf16, tag="pt", name="PT"
                        )
                        tasks.append(make_task(dr, strip, PT))

                if CFG["sw_pipe"]:
                    tasks[0][0]()
                    for i in range(len(tasks)):
                        if i + 1 < len(tasks):
                            tasks[i + 1][0]()
                        tasks[i][1]()
                else:
                    for s, pv in tasks:
                        s()
                        pv()

                nc.sync.dma_start(
                    out=o[h].rearrange("(c p) d -> p c d", p=128), in_=ostage
                )

    nc.compile()
    return nc


def _get_nc():
    if "nc" not in _CACHE:
        _CACHE["nc"] = _build()
    return _CACHE["nc"]


def _make_in_maps(query, key, value):
    import ml_dtypes

    q = query.reshape(B, H, L, D)[:, :, P_OF_PI, :]
    k = key.reshape(B, H, L, D)[:, :, P_OF_PI, :]
    v = value.reshape(B, H, L, D)[:, :, P_OF_PI, :]
    kT = np.ascontiguousarray(k.transpose(0, 1, 3, 2))           # [B,H,D,L]
    vb = np.ascontiguousarray(v).astype(ml_dtypes.bfloat16)      # [B,H,L,D]
    in_maps = []
    for c in range(8):
        b, qh = c // 2, c % 2
        qp = np.empty((H, QROWS, D), np.float32)
        for h in range(H):
            for dr in DRS:
                Lg = L // dr
                off = _off(dr, h)
                lo = off + qh * (Lg // 2)
                qp[h, POFF[dr] : POFF[dr] + Lg // 2] = q[b, h, lo : lo + Lg // 2]
        qpT = np.ascontiguousarray(qp.transpose(0, 2, 1))        # [H,D,QROWS]
        in_maps.append({"qt": qpT, "kt": kT[b], "vb": vb[b]})
    return in_maps


def _assemble(results):
    total_sig = np.zeros((B, H, L, D), np.float32)
    for c in range(8):
        b, qh = c // 2, c % 2
        oc = results[c]["o"]
        for h in range(H):
            for dr in DRS:
                Lg = L // dr
                off = _off(dr, h)
                lo = off + qh * (Lg // 2)
                total_sig[b, h, lo : lo + Lg // 2] += oc[
                    h, POFF[dr] : POFF[dr] + Lg // 2
                ]
    total = total_sig[:, :, SIG, :]
    return np.ascontiguousarray(
        total.transpose(0, 2, 1, 3).reshape(B, L, H * D)
    )


def _run(query, key, value, trace=False, **trace_kwargs):
    from concourse.bass_utils import run_bass_kernel_spmd

    nc = _get_nc()
    in_maps = _make_in_maps(query, key, value)
    res = run_bass_kernel_spmd(
        nc, in_maps, list(range(8)), trace=trace, **trace_kwargs
    )
    return _assemble(res.results), res


def kernel(query, key, value):
    out, _ = _run(query, key, value)
    return out
